# revision 22
# baseline (speedup 1.0000x reference)
"""DNA-structure attention Trainium2 kernel (8-core SPMD).

Reference computation (per batch b):
    qkv = x @ qkv_w.T + qkv_b ; split to q,k,v [H=16 heads, d=64]
    s   = q @ k.T / 8 + dna_bias ; causal mask ; p = softmax(s)
    o   = p @ v ; y = concat_heads(o) @ out_w.T + out_b

Sharding: 8 cores = 4 batches x 2 head-groups (8 heads each).
Each core computes its batch's partial output y_partial = o_g @ out_w[:, cols_g].T;
host sums the two partials per batch and adds out_b.

Per core, one fused pipeline over 512-query chunks qc (matmul operands in
bf16, accumulation in fp32 PSUM):
  projections(qc): QT,KT chunks = W @ x.T (dims-major); V chunk = x @ Wv.T,
      stored per head with an appended ones column (V' [keys, 65]) so that
      p @ V' also yields the softmax denominators (row 64).
  attention(qc), per head: sT [128k, 512q] = K-chunk.T-matmul (fp32 psum);
      p = exp(sT) * E as bf16, where E = exp(dna_bias.T) * causal(0/1) is
      host-precomputed (its zeros implement the causal mask, so no
      max-subtraction and no masking pass); O.T [65, 512q] accumulates over
      key chunks; normalize by the row-64 reciprocal.
  out-projection(qc) -> partial y, summed (plus out_b) on the host.
Causality: only key chunks kc*128 <= qc*512+511 are computed at all.
"""

import sys

if "/opt/trn_rl_repo" not in sys.path:
    sys.path.insert(0, "/opt/trn_rl_repo")

import numpy as np

import concourse.bass as bass
import concourse.mybir as mybir
import concourse.tile as tile
from concourse import bacc
from concourse.bass_utils import run_bass_kernel_spmd

# The axon NTFF-profiling hook lives in trn_agent_boot in this container but
# concourse expects it at antenv.axon_hooks (absent). Register a shim module
# so run_bass_kernel_spmd(trace=True) can capture real HW timings.
if "antenv.axon_hooks" not in sys.modules:
    import types

    def _get_axon_ntff_profile_hook(_cache=[]):
        if not _cache:
            try:
                from trn_agent_boot.trn_boot import _ntff_profile_via_ctypes
                _cache.append(
                    _ntff_profile_via_ctypes("/opt/axon/libaxon_pjrt.so"))
            except Exception:
                _cache.append(None)
        return _cache[0]

    _m = types.ModuleType("antenv.axon_hooks")
    _m.get_axon_ntff_profile_hook = _get_axon_ntff_profile_hook
    sys.modules["antenv.axon_hooks"] = _m

B, T, DIM = 4, 2048, 1024
HEADS = 16
HD = 64  # head dim
N_CORES = 8
HPC = 8            # heads per core
CPC = HPC * HD     # channel slice per core (512)
QC = 512           # query chunk
N_QC = T // QC     # 4
KC = 128           # key chunk (partition dim of scores_T tiles)
N_KC = T // KC     # 16
P = 128

F32 = mybir.dt.float32
BF16 = mybir.dt.bfloat16


def build_program():
    nc = bacc.Bacc("TRN2", target_bir_lowering=False, debug=False,
                   num_devices=N_CORES)

    x_t = nc.declare_dram_parameter("x_t", [DIM, T], BF16, isOutput=False)
    wq_t = nc.declare_dram_parameter("wq_t", [DIM, CPC], BF16, isOutput=False)
    wk_t = nc.declare_dram_parameter("wk_t", [DIM, CPC], BF16, isOutput=False)
    wv_t = nc.declare_dram_parameter("wv_t", [DIM, CPC], BF16, isOutput=False)
    bq = nc.declare_dram_parameter("bq", [CPC, 1], F32, isOutput=False)
    bk = nc.declare_dram_parameter("bk", [CPC, 1], F32, isOutput=False)
    bv = nc.declare_dram_parameter("bv", [P, CPC], F32, isOutput=False)
    wo_t = nc.declare_dram_parameter("wo_t", [CPC, DIM], BF16, isOutput=False)
    e_t = nc.declare_dram_parameter("e_t", [T, T], BF16, isOutput=False)
    y = nc.declare_dram_parameter("y", [T, DIM], F32, isOutput=True)

    NCC = DIM // P    # 8 contraction chunks for qkv projection
    NQD = CPC // P    # 4 dim-chunks of Q/K

    with tile.TileContext(nc) as tc:
        with (
            tc.tile_pool(name="persist", bufs=1) as persist,
            tc.tile_pool(name="wts", bufs=1) as wts,
            tc.tile_pool(name="xw", bufs=10) as xw,
            tc.tile_pool(name="ebuf", bufs=2) as ebuf,
            tc.tile_pool(name="otile", bufs=2) as otile,
            tc.tile_pool(name="wkp", bufs=4) as wkp,
            tc.tile_pool(name="wko", bufs=2) as wko,
            tc.tile_pool(name="wkn", bufs=3) as wkn,
            tc.tile_pool(name="psA", bufs=1, space="PSUM") as psA,
            tc.tile_pool(name="psS", bufs=2, space="PSUM") as psS,
            tc.tile_pool(name="psO", bufs=2, space="PSUM") as psO,
        ):
            # ---------------- persistent SBUF ----------------
            # Q.T [512 dims, T] (matmul rhs, full 128 partitions).
            qt_buf = persist.tile([P, NQD, T], BF16)
            # K.T per head, zero-padded to the full 128 partitions so the
            # scores matmul streams a full-width moving operand: head h
            # occupies partitions (h%2)*64..+63 of slot h, rest stays zero.
            kt_pad = persist.tile([P, HPC, T], BF16)
            # V' per head, padded to 96 columns (32-aligned): cols 0..63 = V,
            # col 64 = ones (softmax denominator trick), cols 65..95 = zero.
            vp_buf = persist.tile([P, N_KC, HPC, 96], BF16)
            bq_sb = persist.tile([P, NQD], F32)
            bk_sb = persist.tile([P, NQD], F32)
            bv_sb = persist.tile([P, CPC], F32)
            wq_sb = wts.tile([P, NCC, CPC], BF16)
            wk_sb = wts.tile([P, NCC, CPC], BF16)
            wv_sb = wts.tile([P, NCC, CPC], BF16)
            wo_sb = wts.tile([P, NQD, DIM], BF16)      # out_w.T slice

            nc.sync.dma_start(bq_sb[:], bq.rearrange("(c p) o -> p (c o)", p=P))
            nc.sync.dma_start(bk_sb[:], bk.rearrange("(c p) o -> p (c o)", p=P))
            nc.sync.dma_start(bv_sb[:], bv[:])
            nc.gpsimd.memset(kt_pad[:], 0.0)
            nc.gpsimd.memset(vp_buf[:], 0.0)
            nc.gpsimd.memset(vp_buf[:, :, :, 64:65], 1.0)

            x_t3 = x_t.rearrange("(c p) t -> p c t", p=P)
            e_t3 = e_t.rearrange("(ko p) t -> p ko t", p=P)
            wq_t3 = wq_t.rearrange("(c p) m -> p c m", p=P)

            QW = 2 * QC               # projection window (1024 tokens)

            def stage_xts(qp):
                tw = slice(qp * QW, (qp + 1) * QW)
                xts = []
                for cc in range(NCC):
                    xt = xw.tile([P, QW], BF16, tag="xt")
                    nc.sync.dma_start(xt[:], x_t3[:, cc, tw])
                    xts.append(xt)
                return xts

            # DMA order matters for the PE start-up stall: the first
            # matmuls need wq chunk cc + x tile cc, in cc order.
            xts0 = None
            for cc in range(NCC):
                nc.sync.dma_start(wq_sb[:, cc, :], wq_t3[:, cc, :])
                if cc == 0:
                    xts0 = stage_xts(0)
            nc.sync.dma_start(wk_sb[:], wk_t.rearrange("(c p) m -> p c m", p=P))
            nc.sync.dma_start(wv_sb[:], wv_t.rearrange("(c p) m -> p c m", p=P))
            nc.sync.dma_start(wo_sb[:], wo_t.rearrange("(c p) d -> p c d", p=P))

            def projections(qp, xts):
                tw = slice(qp * QW, (qp + 1) * QW)
                # Q.T: out [dims 128, 1024] = Wq.T-chunk.T @ x.T-chunk
                for qd in range(NQD):
                    ps = psA.tile([P, QW], F32, tag="psA")
                    for half in range(2):
                        hq = slice(half * QC, (half + 1) * QC)
                        for cc in range(NCC):
                            nc.tensor.matmul(
                                ps[:, hq],
                                wq_sb[:, cc, qd * P:(qd + 1) * P],
                                xts[cc][:, hq],
                                start=(cc == 0), stop=(cc == NCC - 1),
                            )
                    nc.vector.tensor_tensor(
                        qt_buf[:, qd, tw], ps[:],
                        bq_sb[:, qd:qd + 1].to_broadcast([P, QW]),
                        mybir.AluOpType.add,
                    )
                # K.T into the per-head zero-padded layout
                for hc in range(NQD):
                    ps = psA.tile([P, QW], F32, tag="psA")
                    for half in range(2):
                        hq = slice(half * QC, (half + 1) * QC)
                        for cc in range(NCC):
                            nc.tensor.matmul(
                                ps[:, hq],
                                wk_sb[:, cc, hc * P:(hc + 1) * P],
                                xts[cc][:, hq],
                                start=(cc == 0), stop=(cc == NCC - 1),
                            )
                    for half in range(2):
                        hsl = slice(half * HD, half * HD + HD)
                        nc.vector.tensor_tensor(
                            kt_pad[hsl, 2 * hc + half, tw], ps[hsl, :],
                            bk_sb[hsl, hc:hc + 1].to_broadcast([HD, QW]),
                            mybir.AluOpType.add,
                        )
                # V: out [keys 128, 512 dims] = x.T-chunk.T @ Wv.T-chunk
                for ts_ in range(QW // P):
                    kc_idx = qp * (QW // P) + ts_
                    ps = psA.tile([P, QW], F32, tag="psA")
                    for cc in range(NCC):
                        nc.tensor.matmul(
                            ps[:, 0:CPC],
                            xts[cc][:, ts_ * P:(ts_ + 1) * P],
                            wv_sb[:, cc, :],
                            start=(cc == 0), stop=(cc == NCC - 1),
                        )
                    nc.vector.tensor_tensor(
                        vp_buf[:, kc_idx, :, 0:HD],
                        ps[:, 0:CPC].rearrange("p (h d) -> p h d", d=HD),
                        bv_sb.rearrange("p (h d) -> p h d", d=HD),
                        mybir.AluOpType.add,
                    )

            mult_i = 0  # round-robin counter for the DVE/GpSimd split

            def attention(qc):
                nonlocal mult_i
                tq = slice(qc * QC, (qc + 1) * QC)
                n_kc = (qc + 1) * (QC // KC)      # causal: key chunks used
                e_sb = ebuf.tile([P, N_KC, QC], BF16, tag="e")
                for kp2 in range(n_kc // 2):
                    nc.sync.dma_start(
                        e_sb[:, 2 * kp2:2 * kp2 + 2, :],
                        e_t3[:, 2 * kp2:2 * kp2 + 2, tq])

                ot_buf = otile.tile([P, NQD, QC], BF16, tag="ot")

                def normalize(h, o_ps):
                    # denominators sit in row HD of the accumulator
                    hp = (h % 2) * HD
                    hc = h // 2
                    r_sb = wkn.tile([1, QC], F32, tag="r")
                    nc.vector.reciprocal(r_sb[:], o_ps[HD:HD + 1, :])
                    rr_sb = wkn.tile([HD, QC], F32, tag="rr")
                    nc.gpsimd.partition_broadcast(rr_sb[:], r_sb[:])
                    nc.vector.tensor_tensor(
                        ot_buf[hp:hp + HD, hc, :], o_ps[0:HD, :],
                        rr_sb[:],
                        mybir.AluOpType.mult)

                pending = None        # (h, o_ps) whose normalize is deferred
                for h in range(HPC):
                    hp = (h % 2) * HD        # partition offset inside chunk
                    hc = h // 2              # which 128-dim chunk
                    o_ps = psO.tile([96, QC], F32, tag="o")
                    # Key chunks in pairs: one [128, 2*QC] psum tile
                    # (2 banks) -> single fat exp + single fat mult.
                    # attn@v is emitted one pair behind (software
                    # pipelining) so PE is not blocked on exp+mult latency.
                    n_kp = n_kc // 2
                    p_tiles = [None] * n_kp

                    def emit_av(kp, o_ps=o_ps, h=h, n_kc=n_kc,
                                p_tiles=p_tiles):
                        for j in range(2):
                            kc = 2 * kp + j
                            nc.tensor.matmul(
                                o_ps[:],
                                vp_buf[:, kc, h, :],
                                p_tiles[kp][:, j * QC:(j + 1) * QC],
                                start=(kc == 0), stop=(kc == n_kc - 1),
                            )
                        p_tiles[kp] = None

                    for kp in range(n_kp):
                        s_ps = psS.tile([P, 2 * QC], F32, tag="s")
                        for j in range(2):
                            kc = 2 * kp + j
                            nc.tensor.matmul(
                                s_ps[:, j * QC:(j + 1) * QC],
                                kt_pad[:, h, kc * KC:(kc + 1) * KC],
                                qt_buf[:, hc, tq],
                                start=True, stop=True,
                            )
                        p_sb = wkp.tile([P, 2 * QC], BF16, tag="p")
                        nc.scalar.activation(
                            p_sb[:], s_ps[:],
                            mybir.ActivationFunctionType.Exp)
                        e_pair = e_sb[:, 2 * kp:2 * kp + 2, :] \
                            .rearrange("p a q -> p (a q)")
                        mult_eng = (nc.gpsimd if mult_i % 3 == 2
                                    else nc.vector)
                        mult_i += 1
                        mult_eng.tensor_tensor(
                            p_sb[:], p_sb[:], e_pair,
                            mybir.AluOpType.mult)
                        p_tiles[kp] = p_sb
                        if kp >= 1:
                            emit_av(kp - 1)
                        # normalize the previous head only after this head's
                        # first multiplies are queued, so the reciprocal does
                        # not block the DVE FIFO in front of them
                        if kp == min(1, n_kp - 1) and pending is not None:
                            normalize(*pending)
                            pending = None
                    emit_av(n_kp - 1)
                    pending = (h, o_ps)
                if pending is not None:
                    normalize(*pending)
                return ot_buf

            def out_projection(qc, ot_buf):
                for m in range(QC // P):
                    ps = psA.tile([P, QW], F32, tag="psA")
                    for half in range(2):
                        hn = slice(half * QC, (half + 1) * QC)
                        for cc in range(NQD):
                            nc.tensor.matmul(
                                ps[:, hn],
                                ot_buf[:, cc, m * P:(m + 1) * P],
                                wo_sb[:, cc, hn],
                                start=(cc == 0), stop=(cc == NQD - 1),
                            )
                    o_sb = wko.tile([P, DIM], F32, tag="osb")
                    nc.vector.tensor_copy(o_sb[:], ps[:, 0:DIM])
                    nc.sync.dma_start(
                        y[qc * QC + m * P: qc * QC + (m + 1) * P, :],
                        o_sb[:])

            xts_next = xts0
            for qp in range(N_QC // 2):
                xts = xts_next
                projections(qp, xts)
                ot_buf = attention(2 * qp)
                # prefetch the next window's x tiles only now: their WAR wait
                # (on this window's projections) would otherwise head-block
                # the Sync DMA queue in front of the attention E loads
                if qp + 1 < N_QC // 2:
                    xts_next = stage_xts(qp + 1)
                out_projection(2 * qp, ot_buf)
                ot_buf = attention(2 * qp + 1)
                out_projection(2 * qp + 1, ot_buf)

    nc.finalize()
    return nc


_PROGRAM = None


def _get_program():
    global _PROGRAM
    if _PROGRAM is None:
        _PROGRAM = build_program()
    return _PROGRAM


def _bf16(a):
    import ml_dtypes
    return np.ascontiguousarray(np.asarray(a, np.float32)).astype(
        ml_dtypes.bfloat16)


def make_in_maps(x, qkv_w, qkv_b, out_w, out_b, dna_bias):
    x = np.asarray(x, np.float32)
    qkv_w = np.asarray(qkv_w, np.float32)
    qkv_b = np.asarray(qkv_b, np.float32)
    out_w = np.asarray(out_w, np.float32)
    dna_bias = np.asarray(dna_bias, np.float32)

    scale = 1.0 / np.sqrt(HD)
    bias = dna_bias[:T, :T]
    causal = np.tril(np.ones((T, T), np.float32))
    e_t = _bf16((np.exp(bias) * causal).T)

    in_maps = []
    for core in range(N_CORES):
        b, g = divmod(core, 2)
        cols = slice(g * CPC, (g + 1) * CPC)
        wq = qkv_w[0 * DIM:1 * DIM][cols] * scale      # [512, 1024]
        wk = qkv_w[1 * DIM:2 * DIM][cols]
        wv = qkv_w[2 * DIM:3 * DIM][cols]
        in_maps.append({
            "x_t": _bf16(x[b].T),
            "wq_t": _bf16(wq.T),
            "wk_t": _bf16(wk.T),
            "wv_t": _bf16(wv.T),
            "bq": np.ascontiguousarray(
                (qkv_b[0 * DIM:1 * DIM][cols] * scale)[:, None]),
            "bk": np.ascontiguousarray(qkv_b[1 * DIM:2 * DIM][cols][:, None]),
            "bv": np.ascontiguousarray(
                np.broadcast_to(qkv_b[2 * DIM:3 * DIM][cols][None, :],
                                (P, CPC))),
            "wo_t": _bf16(out_w[:, cols].T),
            "e_t": e_t,
        })
    return in_maps


LAST_RESULTS = None


def kernel(x, qkv_w, qkv_b, out_w, out_b, dna_bias, **run_kwargs):
    global LAST_RESULTS
    nc = _get_program()
    in_maps = make_in_maps(x, qkv_w, qkv_b, out_w, out_b, dna_bias)
    res = run_bass_kernel_spmd(nc, in_maps, list(range(N_CORES)), **run_kwargs)
    LAST_RESULTS = res
    out_b = np.asarray(out_b, np.float32)
    out = np.empty((B, T, DIM), np.float32)
    for b in range(B):
        out[b] = res.results[2 * b]["y"] + res.results[2 * b + 1]["y"] + out_b
    return out


# revision 23
# speedup vs baseline: 1.0699x; 1.0699x over previous
"""DNA-structure attention Trainium2 kernel (8-core SPMD).

Reference computation (per batch b):
    qkv = x @ qkv_w.T + qkv_b ; split to q,k,v [H=16 heads, d=64]
    s   = q @ k.T / 8 + dna_bias ; causal mask ; p = softmax(s)
    o   = p @ v ; y = concat_heads(o) @ out_w.T + out_b

Sharding: 8 cores = 4 batches x 2 head-groups (8 heads each).
Each core computes its batch's partial output y_partial = o_g @ out_w[:, cols_g].T;
host sums the two partials per batch and adds out_b.

Per core, one fused pipeline over 512-query chunks qc (matmul operands in
bf16, accumulation in fp32 PSUM):
  projections(qc): QT,KT chunks = W @ x.T (dims-major); V chunk = x @ Wv.T,
      stored per head with an appended ones column (V' [keys, 65]) so that
      p @ V' also yields the softmax denominators (row 64).
  attention(qc), per head: sT [128k, 512q] = K-chunk.T-matmul (fp32 psum);
      p = exp(sT) * E as bf16, where E = exp(dna_bias.T) * causal(0/1) is
      host-precomputed (its zeros implement the causal mask, so no
      max-subtraction and no masking pass); O.T [65, 512q] accumulates over
      key chunks; normalize by the row-64 reciprocal.
  out-projection(qc) -> partial y, summed (plus out_b) on the host.
Causality: only key chunks kc*128 <= qc*512+511 are computed at all.
"""

import sys

if "/opt/trn_rl_repo" not in sys.path:
    sys.path.insert(0, "/opt/trn_rl_repo")

import numpy as np

import concourse.bass as bass
import concourse.mybir as mybir
import concourse.tile as tile
from concourse import bacc
from concourse.bass_utils import run_bass_kernel_spmd

# The axon NTFF-profiling hook lives in trn_agent_boot in this container but
# concourse expects it at antenv.axon_hooks (absent). Register a shim module
# so run_bass_kernel_spmd(trace=True) can capture real HW timings.
if "antenv.axon_hooks" not in sys.modules:
    import types

    def _get_axon_ntff_profile_hook(_cache=[]):
        if not _cache:
            try:
                from trn_agent_boot.trn_boot import _ntff_profile_via_ctypes
                _cache.append(
                    _ntff_profile_via_ctypes("/opt/axon/libaxon_pjrt.so"))
            except Exception:
                _cache.append(None)
        return _cache[0]

    _m = types.ModuleType("antenv.axon_hooks")
    _m.get_axon_ntff_profile_hook = _get_axon_ntff_profile_hook
    sys.modules["antenv.axon_hooks"] = _m

B, T, DIM = 4, 2048, 1024
HEADS = 16
HD = 64  # head dim
N_CORES = 8
HPC = 8            # heads per core
CPC = HPC * HD     # channel slice per core (512)
QC = 512           # query chunk
N_QC = T // QC     # 4
KC = 128           # key chunk (partition dim of scores_T tiles)
N_KC = T // KC     # 16
P = 128

F32 = mybir.dt.float32
BF16 = mybir.dt.bfloat16


def build_program():
    nc = bacc.Bacc("TRN2", target_bir_lowering=False, debug=False,
                   num_devices=N_CORES)

    x_t = nc.declare_dram_parameter("x_t", [DIM, T], BF16, isOutput=False)
    wq_t = nc.declare_dram_parameter("wq_t", [DIM, CPC], BF16, isOutput=False)
    wk_t = nc.declare_dram_parameter("wk_t", [DIM, CPC], BF16, isOutput=False)
    wv_t = nc.declare_dram_parameter("wv_t", [DIM, CPC], BF16, isOutput=False)
    bq = nc.declare_dram_parameter("bq", [CPC, 1], F32, isOutput=False)
    bk = nc.declare_dram_parameter("bk", [CPC, 1], F32, isOutput=False)
    bv = nc.declare_dram_parameter("bv", [P, CPC], F32, isOutput=False)
    wo_t = nc.declare_dram_parameter("wo_t", [CPC, DIM], BF16, isOutput=False)
    e_t = nc.declare_dram_parameter("e_t", [T, T], BF16, isOutput=False)
    y = nc.declare_dram_parameter("y", [T, DIM], F32, isOutput=True)

    NCC = DIM // P    # 8 contraction chunks for qkv projection
    NQD = CPC // P    # 4 dim-chunks of Q/K

    with tile.TileContext(nc) as tc:
        with (
            tc.tile_pool(name="persist", bufs=1) as persist,
            tc.tile_pool(name="wts", bufs=1) as wts,
            tc.tile_pool(name="xw", bufs=10) as xw,
            tc.tile_pool(name="ebuf", bufs=2) as ebuf,
            tc.tile_pool(name="otile", bufs=2) as otile,
            tc.tile_pool(name="wkp", bufs=4) as wkp,
            tc.tile_pool(name="wko", bufs=2) as wko,
            tc.tile_pool(name="wkn", bufs=3) as wkn,
            tc.tile_pool(name="psA", bufs=1, space="PSUM") as psA,
            tc.tile_pool(name="psS", bufs=2, space="PSUM") as psS,
            tc.tile_pool(name="psO", bufs=2, space="PSUM") as psO,
        ):
            # ---------------- persistent SBUF ----------------
            # Q.T [512 dims, T] (matmul rhs, full 128 partitions).
            qt_buf = persist.tile([P, NQD, T], BF16)
            # K.T per head, zero-padded to the full 128 partitions so the
            # scores matmul streams a full-width moving operand: head h
            # occupies partitions (h%2)*64..+63 of slot h, rest stays zero.
            kt_pad = persist.tile([P, HPC, T], BF16)
            # V' per head, padded to 96 columns (32-aligned): cols 0..63 = V,
            # col 64 = ones (softmax denominator trick), cols 65..95 = zero.
            vp_buf = persist.tile([P, N_KC, HPC, 96], BF16)
            bq_sb = persist.tile([P, NQD], F32)
            bk_sb = persist.tile([P, NQD], F32)
            bv_sb = persist.tile([P, CPC], F32)
            wq_sb = wts.tile([P, NCC, CPC], BF16)
            wk_sb = wts.tile([P, NCC, CPC], BF16)
            wv_sb = wts.tile([P, NCC, CPC], BF16)
            wo_sb = wts.tile([P, NQD, DIM], BF16)      # out_w.T slice

            nc.sync.dma_start(bq_sb[:], bq.rearrange("(c p) o -> p (c o)", p=P))
            nc.sync.dma_start(bk_sb[:], bk.rearrange("(c p) o -> p (c o)", p=P))
            nc.sync.dma_start(bv_sb[:], bv[:])
            nc.gpsimd.memset(kt_pad[:], 0.0)
            nc.gpsimd.memset(vp_buf[:], 0.0)
            nc.gpsimd.memset(vp_buf[:, :, :, 64:65], 1.0)

            x_t3 = x_t.rearrange("(c p) t -> p c t", p=P)
            e_t3 = e_t.rearrange("(ko p) t -> p ko t", p=P)
            wq_t3 = wq_t.rearrange("(c p) m -> p c m", p=P)

            QW = 2 * QC               # projection window (1024 tokens)

            def stage_xts(qp):
                tw = slice(qp * QW, (qp + 1) * QW)
                xts = []
                for cc in range(NCC):
                    xt = xw.tile([P, QW], BF16, tag="xt")
                    nc.sync.dma_start(xt[:], x_t3[:, cc, tw])
                    xts.append(xt)
                return xts

            # DMA order matters for the PE start-up stall: the first
            # matmuls need wq chunk cc + x tile cc, in cc order.
            xts0 = None
            for cc in range(NCC):
                nc.sync.dma_start(wq_sb[:, cc, :], wq_t3[:, cc, :])
                if cc == 0:
                    xts0 = stage_xts(0)
            nc.sync.dma_start(wk_sb[:], wk_t.rearrange("(c p) m -> p c m", p=P))
            nc.sync.dma_start(wv_sb[:], wv_t.rearrange("(c p) m -> p c m", p=P))
            nc.sync.dma_start(wo_sb[:], wo_t.rearrange("(c p) d -> p c d", p=P))

            def projections(qp, xts):
                tw = slice(qp * QW, (qp + 1) * QW)
                # Q.T: out [dims 128, 1024] = Wq.T-chunk.T @ x.T-chunk
                for qd in range(NQD):
                    ps = psA.tile([P, QW], F32, tag="psA")
                    for half in range(2):
                        hq = slice(half * QC, (half + 1) * QC)
                        for cc in range(NCC):
                            nc.tensor.matmul(
                                ps[:, hq],
                                wq_sb[:, cc, qd * P:(qd + 1) * P],
                                xts[cc][:, hq],
                                start=(cc == 0), stop=(cc == NCC - 1),
                            )
                    nc.vector.tensor_tensor(
                        qt_buf[:, qd, tw], ps[:],
                        bq_sb[:, qd:qd + 1].to_broadcast([P, QW]),
                        mybir.AluOpType.add,
                    )
                # K.T into the per-head zero-padded layout
                for hc in range(NQD):
                    ps = psA.tile([P, QW], F32, tag="psA")
                    for half in range(2):
                        hq = slice(half * QC, (half + 1) * QC)
                        for cc in range(NCC):
                            nc.tensor.matmul(
                                ps[:, hq],
                                wk_sb[:, cc, hc * P:(hc + 1) * P],
                                xts[cc][:, hq],
                                start=(cc == 0), stop=(cc == NCC - 1),
                            )
                    for half in range(2):
                        hsl = slice(half * HD, half * HD + HD)
                        nc.vector.tensor_tensor(
                            kt_pad[hsl, 2 * hc + half, tw], ps[hsl, :],
                            bk_sb[hsl, hc:hc + 1].to_broadcast([HD, QW]),
                            mybir.AluOpType.add,
                        )
                # V: out [keys 128, 512 dims] = x.T-chunk.T @ Wv.T-chunk
                for ts_ in range(QW // P):
                    kc_idx = qp * (QW // P) + ts_
                    ps = psA.tile([P, QW], F32, tag="psA")
                    for cc in range(NCC):
                        nc.tensor.matmul(
                            ps[:, 0:CPC],
                            xts[cc][:, ts_ * P:(ts_ + 1) * P],
                            wv_sb[:, cc, :],
                            start=(cc == 0), stop=(cc == NCC - 1),
                        )
                    nc.vector.tensor_tensor(
                        vp_buf[:, kc_idx, :, 0:HD],
                        ps[:, 0:CPC].rearrange("p (h d) -> p h d", d=HD),
                        bv_sb.rearrange("p (h d) -> p h d", d=HD),
                        mybir.AluOpType.add,
                    )

            mult_i = 0  # round-robin counter for the DVE/GpSimd split

            def attention(qc):
                nonlocal mult_i
                tq = slice(qc * QC, (qc + 1) * QC)
                n_kc = (qc + 1) * (QC // KC)      # causal: key chunks used
                e_sb = ebuf.tile([P, N_KC, QC], BF16, tag="e")
                for kp2 in range(n_kc // 2):
                    nc.sync.dma_start(
                        e_sb[:, 2 * kp2:2 * kp2 + 2, :],
                        e_t3[:, 2 * kp2:2 * kp2 + 2, tq])

                ot_buf = otile.tile([P, NQD, QC], BF16, tag="ot")

                def normalize(h, o_ps):
                    # denominators sit in row HD of the accumulator
                    hp = (h % 2) * HD
                    hc = h // 2
                    r_sb = wkn.tile([1, QC], F32, tag="r")
                    nc.vector.reciprocal_approx_fast(r_sb[:],
                                                     o_ps[HD:HD + 1, :])
                    rr_sb = wkn.tile([HD, QC], F32, tag="rr")
                    nc.gpsimd.partition_broadcast(rr_sb[:], r_sb[:])
                    nc.vector.tensor_tensor(
                        ot_buf[hp:hp + HD, hc, :], o_ps[0:HD, :],
                        rr_sb[:],
                        mybir.AluOpType.mult)

                pending = None        # (h, o_ps) whose normalize is deferred
                for h in range(HPC):
                    hp = (h % 2) * HD        # partition offset inside chunk
                    hc = h // 2              # which 128-dim chunk
                    o_ps = psO.tile([96, QC], F32, tag="o")
                    # Key chunks in pairs: one [128, 2*QC] psum tile
                    # (2 banks) -> single fat exp + single fat mult.
                    # attn@v is emitted one pair behind (software
                    # pipelining) so PE is not blocked on exp+mult latency.
                    n_kp = n_kc // 2
                    p_tiles = [None] * n_kp

                    def emit_av(kp, o_ps=o_ps, h=h, n_kc=n_kc,
                                p_tiles=p_tiles):
                        for j in range(2):
                            kc = 2 * kp + j
                            nc.tensor.matmul(
                                o_ps[:],
                                vp_buf[:, kc, h, :],
                                p_tiles[kp][:, j * QC:(j + 1) * QC],
                                start=(kc == 0), stop=(kc == n_kc - 1),
                            )
                        p_tiles[kp] = None

                    for kp in range(n_kp):
                        s_ps = psS.tile([P, 2 * QC], F32, tag="s")
                        for j in range(2):
                            kc = 2 * kp + j
                            nc.tensor.matmul(
                                s_ps[:, j * QC:(j + 1) * QC],
                                kt_pad[:, h, kc * KC:(kc + 1) * KC],
                                qt_buf[:, hc, tq],
                                start=True, stop=True,
                            )
                        p_sb = wkp.tile([P, 2 * QC], BF16, tag="p")
                        nc.scalar.activation(
                            p_sb[:], s_ps[:],
                            mybir.ActivationFunctionType.Exp)
                        e_pair = e_sb[:, 2 * kp:2 * kp + 2, :] \
                            .rearrange("p a q -> p (a q)")
                        mult_i += 1
                        nc.vector.tensor_tensor(
                            p_sb[:], p_sb[:], e_pair,
                            mybir.AluOpType.mult)
                        p_tiles[kp] = p_sb
                        if kp >= 1:
                            emit_av(kp - 1)
                        # normalize the previous head only after this head's
                        # first multiplies are queued, so the reciprocal does
                        # not block the DVE FIFO in front of them
                        if kp == min(1, n_kp - 1) and pending is not None:
                            normalize(*pending)
                            pending = None
                    emit_av(n_kp - 1)
                    pending = (h, o_ps)
                if pending is not None:
                    normalize(*pending)
                return ot_buf

            def out_projection(qc, ot_buf):
                for m in range(QC // P):
                    ps = psA.tile([P, QW], F32, tag="psA")
                    for half in range(2):
                        hn = slice(half * QC, (half + 1) * QC)
                        for cc in range(NQD):
                            nc.tensor.matmul(
                                ps[:, hn],
                                ot_buf[:, cc, m * P:(m + 1) * P],
                                wo_sb[:, cc, hn],
                                start=(cc == 0), stop=(cc == NQD - 1),
                            )
                    o_sb = wko.tile([P, DIM], F32, tag="osb")
                    nc.vector.tensor_copy(o_sb[:], ps[:, 0:DIM])
                    nc.sync.dma_start(
                        y[qc * QC + m * P: qc * QC + (m + 1) * P, :],
                        o_sb[:])

            xts_next = xts0
            for qp in range(N_QC // 2):
                xts = xts_next
                projections(qp, xts)
                ot_buf = attention(2 * qp)
                # prefetch the next window's x tiles only now: their WAR wait
                # (on this window's projections) would otherwise head-block
                # the Sync DMA queue in front of the attention E loads
                if qp + 1 < N_QC // 2:
                    xts_next = stage_xts(qp + 1)
                out_projection(2 * qp, ot_buf)
                ot_buf = attention(2 * qp + 1)
                out_projection(2 * qp + 1, ot_buf)

    nc.finalize()
    return nc


_PROGRAM = None


def _get_program():
    global _PROGRAM
    if _PROGRAM is None:
        _PROGRAM = build_program()
    return _PROGRAM


def _bf16(a):
    import ml_dtypes
    return np.ascontiguousarray(np.asarray(a, np.float32)).astype(
        ml_dtypes.bfloat16)


def make_in_maps(x, qkv_w, qkv_b, out_w, out_b, dna_bias):
    x = np.asarray(x, np.float32)
    qkv_w = np.asarray(qkv_w, np.float32)
    qkv_b = np.asarray(qkv_b, np.float32)
    out_w = np.asarray(out_w, np.float32)
    dna_bias = np.asarray(dna_bias, np.float32)

    scale = 1.0 / np.sqrt(HD)
    bias = dna_bias[:T, :T]
    causal = np.tril(np.ones((T, T), np.float32))
    e_t = _bf16((np.exp(bias) * causal).T)

    in_maps = []
    for core in range(N_CORES):
        b, g = divmod(core, 2)
        cols = slice(g * CPC, (g + 1) * CPC)
        wq = qkv_w[0 * DIM:1 * DIM][cols] * scale      # [512, 1024]
        wk = qkv_w[1 * DIM:2 * DIM][cols]
        wv = qkv_w[2 * DIM:3 * DIM][cols]
        in_maps.append({
            "x_t": _bf16(x[b].T),
            "wq_t": _bf16(wq.T),
            "wk_t": _bf16(wk.T),
            "wv_t": _bf16(wv.T),
            "bq": np.ascontiguousarray(
                (qkv_b[0 * DIM:1 * DIM][cols] * scale)[:, None]),
            "bk": np.ascontiguousarray(qkv_b[1 * DIM:2 * DIM][cols][:, None]),
            "bv": np.ascontiguousarray(
                np.broadcast_to(qkv_b[2 * DIM:3 * DIM][cols][None, :],
                                (P, CPC))),
            "wo_t": _bf16(out_w[:, cols].T),
            "e_t": e_t,
        })
    return in_maps


LAST_RESULTS = None


def kernel(x, qkv_w, qkv_b, out_w, out_b, dna_bias, **run_kwargs):
    global LAST_RESULTS
    nc = _get_program()
    in_maps = make_in_maps(x, qkv_w, qkv_b, out_w, out_b, dna_bias)
    res = run_bass_kernel_spmd(nc, in_maps, list(range(N_CORES)), **run_kwargs)
    LAST_RESULTS = res
    out_b = np.asarray(out_b, np.float32)
    out = np.empty((B, T, DIM), np.float32)
    for b in range(B):
        out[b] = res.results[2 * b]["y"] + res.results[2 * b + 1]["y"] + out_b
    return out


# revision 25
# speedup vs baseline: 1.7734x; 1.6576x over previous
"""DNA-structure attention Trainium2 kernel (8-core SPMD).

Reference computation (per batch b):
    qkv = x @ qkv_w.T + qkv_b ; split to q,k,v [H=16 heads, d=64]
    s   = q @ k.T / 8 + dna_bias ; causal mask ; p = softmax(s)
    o   = p @ v ; y = concat_heads(o) @ out_w.T + out_b

Sharding: 8 cores = 4 batches x 2 head-groups (8 heads each).
Each core computes its batch's partial output y_partial = o_g @ out_w[:, cols_g].T;
host sums the two partials per batch and adds out_b.

Per core, one fused pipeline over 512-query chunks qc (matmul operands in
bf16, accumulation in fp32 PSUM):
  projections(qc): QT,KT chunks = W @ x.T (dims-major); V chunk = x @ Wv.T,
      stored per head with an appended ones column (V' [keys, 65]) so that
      p @ V' also yields the softmax denominators (row 64).
  attention(qc), per head: sT [128k, 512q] = K-chunk.T-matmul (fp32 psum);
      p = exp(sT) * E as bf16, where E = exp(dna_bias.T) * causal(0/1) is
      host-precomputed (its zeros implement the causal mask, so no
      max-subtraction and no masking pass); O.T [65, 512q] accumulates over
      key chunks; normalize by the row-64 reciprocal.
  out-projection(qc) -> partial y, summed (plus out_b) on the host.
Causality: only key chunks kc*128 <= qc*512+511 are computed at all.
"""

import sys

if "/opt/trn_rl_repo" not in sys.path:
    sys.path.insert(0, "/opt/trn_rl_repo")

import numpy as np

import concourse.bass as bass
import concourse.mybir as mybir
import concourse.tile as tile
from concourse import bacc
from concourse.bass_utils import run_bass_kernel_spmd

# The axon NTFF-profiling hook lives in trn_agent_boot in this container but
# concourse expects it at antenv.axon_hooks (absent). Register a shim module
# so run_bass_kernel_spmd(trace=True) can capture real HW timings.
if "antenv.axon_hooks" not in sys.modules:
    import types

    def _get_axon_ntff_profile_hook(_cache=[]):
        if not _cache:
            try:
                from trn_agent_boot.trn_boot import _ntff_profile_via_ctypes
                _cache.append(
                    _ntff_profile_via_ctypes("/opt/axon/libaxon_pjrt.so"))
            except Exception:
                _cache.append(None)
        return _cache[0]

    _m = types.ModuleType("antenv.axon_hooks")
    _m.get_axon_ntff_profile_hook = _get_axon_ntff_profile_hook
    sys.modules["antenv.axon_hooks"] = _m

B, T, DIM = 4, 2048, 1024
HEADS = 16
HD = 64  # head dim
N_CORES = 8
HPC = 8            # heads per core
CPC = HPC * HD     # channel slice per core (512)
QC = 512           # query chunk
N_QC = T // QC     # 4
KC = 128           # key chunk (partition dim of scores_T tiles)
N_KC = T // KC     # 16
P = 128

F32 = mybir.dt.float32
BF16 = mybir.dt.bfloat16


def build_program():
    nc = bacc.Bacc("TRN2", target_bir_lowering=False, debug=False,
                   num_devices=N_CORES)

    x_t = nc.declare_dram_parameter("x_t", [DIM, T], BF16, isOutput=False)
    wq_t = nc.declare_dram_parameter("wq_t", [DIM, CPC], BF16, isOutput=False)
    wk_t = nc.declare_dram_parameter("wk_t", [DIM, CPC], BF16, isOutput=False)
    wv_t = nc.declare_dram_parameter("wv_t", [DIM, CPC], BF16, isOutput=False)
    bq = nc.declare_dram_parameter("bq", [CPC, 1], F32, isOutput=False)
    bk = nc.declare_dram_parameter("bk", [CPC, 1], F32, isOutput=False)
    bv = nc.declare_dram_parameter("bv", [P, CPC], F32, isOutput=False)
    wo_t = nc.declare_dram_parameter("wo_t", [CPC, DIM], BF16, isOutput=False)
    e_t = nc.declare_dram_parameter("e_t", [T, T], BF16, isOutput=False)
    y = nc.declare_dram_parameter("y", [T, DIM], F32, isOutput=True)

    NCC = DIM // P    # 8 contraction chunks for qkv projection
    NQD = CPC // P    # 4 dim-chunks of Q/K

    with tile.TileContext(nc) as tc:
        with (
            tc.tile_pool(name="persist", bufs=1) as persist,
            tc.tile_pool(name="wts", bufs=1) as wts,
            tc.tile_pool(name="xw", bufs=10) as xw,
            tc.tile_pool(name="ebuf", bufs=2) as ebuf,
            tc.tile_pool(name="otile", bufs=2) as otile,
            tc.tile_pool(name="wkp", bufs=4) as wkp,
            tc.tile_pool(name="wko", bufs=2) as wko,
            tc.tile_pool(name="wkn", bufs=3) as wkn,
            tc.tile_pool(name="psA", bufs=2, space="PSUM") as psA,
            tc.tile_pool(name="psS", bufs=2, space="PSUM") as psS,
            tc.tile_pool(name="psO", bufs=2, space="PSUM") as psO,
        ):
            # ---------------- persistent SBUF ----------------
            # Q.T [512 dims, T] (matmul rhs, full 128 partitions).
            qt_buf = persist.tile([P, NQD, T], BF16)
            # K.T per head, zero-padded to the full 128 partitions so the
            # scores matmul streams a full-width moving operand: head h
            # occupies partitions (h%2)*64..+63 of slot h, rest stays zero.
            kt_pad = persist.tile([P, HPC, T], BF16)
            # V' per head, padded to 96 columns (32-aligned): cols 0..63 = V,
            # col 64 = ones (softmax denominator trick), cols 65..95 = zero.
            vp_buf = persist.tile([P, N_KC, HPC, 96], BF16)
            bq_sb = persist.tile([P, NQD], F32)
            bk_sb = persist.tile([P, NQD], F32)
            bv_sb = persist.tile([P, CPC], F32)
            wq_sb = wts.tile([P, NCC, CPC], BF16)
            wk_sb = wts.tile([P, NCC, CPC], BF16)
            wv_sb = wts.tile([P, NCC, CPC], BF16)
            wo_sb = wts.tile([P, NQD, DIM], BF16)      # out_w.T slice

            nc.sync.dma_start(bq_sb[:], bq.rearrange("(c p) o -> p (c o)", p=P))
            nc.sync.dma_start(bk_sb[:], bk.rearrange("(c p) o -> p (c o)", p=P))
            nc.sync.dma_start(bv_sb[:], bv[:])
            nc.gpsimd.memset(kt_pad[:], 0.0)
            nc.gpsimd.memset(vp_buf[:], 0.0)
            nc.gpsimd.memset(vp_buf[:, :, :, 64:65], 1.0)

            x_t3 = x_t.rearrange("(c p) t -> p c t", p=P)
            e_t3 = e_t.rearrange("(ko p) t -> p ko t", p=P)
            wq_t3 = wq_t.rearrange("(c p) m -> p c m", p=P)

            QW = 2 * QC               # projection window (1024 tokens)

            def stage_xts(qp):
                tw = slice(qp * QW, (qp + 1) * QW)
                xts = []
                for cc in range(NCC):
                    xt = xw.tile([P, QW], BF16, tag="xt")
                    nc.sync.dma_start(xt[:], x_t3[:, cc, tw])
                    xts.append(xt)
                return xts

            # DMA order matters for the PE start-up stall: the first
            # matmuls need wq chunk cc + x tile cc, in cc order.
            xts0 = None
            for cc in range(NCC):
                nc.sync.dma_start(wq_sb[:, cc, :], wq_t3[:, cc, :])
                if cc == 0:
                    xts0 = stage_xts(0)
            nc.sync.dma_start(wk_sb[:], wk_t.rearrange("(c p) m -> p c m", p=P))
            nc.sync.dma_start(wv_sb[:], wv_t.rearrange("(c p) m -> p c m", p=P))
            nc.sync.dma_start(wo_sb[:], wo_t.rearrange("(c p) d -> p c d", p=P))

            def projections(qp, xts):
                tw = slice(qp * QW, (qp + 1) * QW)
                # Q.T: out [dims 128, 1024] = Wq.T-chunk.T @ x.T-chunk
                for qd in range(NQD):
                    for half in range(2):
                        hq = slice(half * QC, (half + 1) * QC)
                        hw_ = slice(qp * QW + half * QC,
                                    qp * QW + (half + 1) * QC)
                        ps = psA.tile([P, QC], F32, tag="psA")
                        for cc in range(NCC):
                            nc.tensor.matmul(
                                ps[:],
                                wq_sb[:, cc, qd * P:(qd + 1) * P],
                                xts[cc][:, hq],
                                start=(cc == 0), stop=(cc == NCC - 1),
                            )
                        nc.vector.tensor_tensor(
                            qt_buf[:, qd, hw_], ps[:],
                            bq_sb[:, qd:qd + 1].to_broadcast([P, QC]),
                            mybir.AluOpType.add,
                        )
                # K.T into the per-head zero-padded layout
                for hc in range(NQD):
                    for half in range(2):
                        hq = slice(half * QC, (half + 1) * QC)
                        hw_ = slice(qp * QW + half * QC,
                                    qp * QW + (half + 1) * QC)
                        ps = psA.tile([P, QC], F32, tag="psA")
                        for cc in range(NCC):
                            nc.tensor.matmul(
                                ps[:],
                                wk_sb[:, cc, hc * P:(hc + 1) * P],
                                xts[cc][:, hq],
                                start=(cc == 0), stop=(cc == NCC - 1),
                            )
                        for hh in range(2):
                            hsl = slice(hh * HD, hh * HD + HD)
                            nc.vector.tensor_tensor(
                                kt_pad[hsl, 2 * hc + hh, hw_], ps[hsl, :],
                                bk_sb[hsl, hc:hc + 1].to_broadcast([HD, QC]),
                                mybir.AluOpType.add,
                            )
                # V: out [keys 128, 512 dims] = x.T-chunk.T @ Wv.T-chunk
                for ts_ in range(QW // P):
                    kc_idx = qp * (QW // P) + ts_
                    ps = psA.tile([P, QC], F32, tag="psA")
                    for cc in range(NCC):
                        nc.tensor.matmul(
                            ps[:],
                            xts[cc][:, ts_ * P:(ts_ + 1) * P],
                            wv_sb[:, cc, :],
                            start=(cc == 0), stop=(cc == NCC - 1),
                        )
                    nc.vector.tensor_tensor(
                        vp_buf[:, kc_idx, :, 0:HD],
                        ps[:].rearrange("p (h d) -> p h d", d=HD),
                        bv_sb.rearrange("p (h d) -> p h d", d=HD),
                        mybir.AluOpType.add,
                    )

            mult_i = 0  # round-robin counter for the DVE/GpSimd split

            def attention(qc):
                nonlocal mult_i
                tq = slice(qc * QC, (qc + 1) * QC)
                n_kc = (qc + 1) * (QC // KC)      # causal: key chunks used
                e_sb = ebuf.tile([P, N_KC, QC], BF16, tag="e")
                for kp2 in range(n_kc // 2):
                    nc.gpsimd.dma_start(
                        e_sb[:, 2 * kp2:2 * kp2 + 2, :],
                        e_t3[:, 2 * kp2:2 * kp2 + 2, tq])

                ot_buf = otile.tile([P, NQD, QC], BF16, tag="ot")

                def normalize(h, o_ps):
                    # denominators sit in row HD of the accumulator
                    hp = (h % 2) * HD
                    hc = h // 2
                    r_sb = wkn.tile([1, QC], F32, tag="r")
                    nc.vector.reciprocal(r_sb[:], o_ps[HD:HD + 1, :])
                    rr_sb = wkn.tile([HD, QC], F32, tag="rr")
                    nc.gpsimd.partition_broadcast(rr_sb[:], r_sb[:])
                    nc.vector.tensor_tensor(
                        ot_buf[hp:hp + HD, hc, :], o_ps[0:HD, :],
                        rr_sb[:],
                        mybir.AluOpType.mult)

                pending = None        # (h, o_ps) whose normalize is deferred
                for h in range(HPC):
                    hp = (h % 2) * HD        # partition offset inside chunk
                    hc = h // 2              # which 128-dim chunk
                    o_ps = psO.tile([96, QC], F32, tag="o")
                    # Key chunks in pairs: one [128, 2*QC] psum tile
                    # (2 banks) -> single fat exp + single fat mult.
                    # attn@v is emitted one pair behind (software
                    # pipelining) so PE is not blocked on exp+mult latency.
                    n_kp = n_kc // 2
                    p_tiles = [None] * n_kp

                    def emit_av(kp, o_ps=o_ps, h=h, n_kc=n_kc,
                                p_tiles=p_tiles):
                        for j in range(2):
                            kc = 2 * kp + j
                            nc.tensor.matmul(
                                o_ps[:],
                                vp_buf[:, kc, h, :],
                                p_tiles[kp][:, j * QC:(j + 1) * QC],
                                start=(kc == 0), stop=(kc == n_kc - 1),
                            )
                        p_tiles[kp] = None

                    for kp in range(n_kp):
                        s_ps = psS.tile([P, 2 * QC], F32, tag="s")
                        for j in range(2):
                            kc = 2 * kp + j
                            nc.tensor.matmul(
                                s_ps[:, j * QC:(j + 1) * QC],
                                kt_pad[:, h, kc * KC:(kc + 1) * KC],
                                qt_buf[:, hc, tq],
                                start=True, stop=True,
                            )
                        p_sb = wkp.tile([P, 2 * QC], BF16, tag="p")
                        nc.scalar.activation(
                            p_sb[:], s_ps[:],
                            mybir.ActivationFunctionType.Exp)
                        e_pair = e_sb[:, 2 * kp:2 * kp + 2, :] \
                            .rearrange("p a q -> p (a q)")
                        mult_i += 1
                        nc.vector.tensor_tensor(
                            p_sb[:], p_sb[:], e_pair,
                            mybir.AluOpType.mult)
                        p_tiles[kp] = p_sb
                        if kp >= 1:
                            emit_av(kp - 1)
                        # normalize the previous head only after this head's
                        # first multiplies are queued, so the reciprocal does
                        # not block the DVE FIFO in front of them
                        if kp == min(1, n_kp - 1) and pending is not None:
                            normalize(*pending)
                            pending = None
                    emit_av(n_kp - 1)
                    pending = (h, o_ps)
                if pending is not None:
                    normalize(*pending)
                return ot_buf

            def out_projection(qc, ot_buf):
                for m in range(QC // P):
                    for half in range(2):
                        hn = slice(half * QC, (half + 1) * QC)
                        ps = psA.tile([P, QC], F32, tag="psA")
                        for cc in range(NQD):
                            nc.tensor.matmul(
                                ps[:],
                                ot_buf[:, cc, m * P:(m + 1) * P],
                                wo_sb[:, cc, hn],
                                start=(cc == 0), stop=(cc == NQD - 1),
                            )
                        o_sb = wko.tile([P, QC], F32, tag="osb")
                        nc.vector.tensor_copy(o_sb[:], ps[:])
                        nc.sync.dma_start(
                            y[qc * QC + m * P: qc * QC + (m + 1) * P, hn],
                            o_sb[:])

            xts_next = xts0
            for qp in range(N_QC // 2):
                xts = xts_next
                projections(qp, xts)
                ot_buf = attention(2 * qp)
                # prefetch the next window's x tiles only now: their WAR wait
                # (on this window's projections) would otherwise head-block
                # the Sync DMA queue in front of the attention E loads
                if qp + 1 < N_QC // 2:
                    xts_next = stage_xts(qp + 1)
                out_projection(2 * qp, ot_buf)
                ot_buf = attention(2 * qp + 1)
                out_projection(2 * qp + 1, ot_buf)

    nc.finalize()
    return nc


_PROGRAM = None


def _get_program():
    global _PROGRAM
    if _PROGRAM is None:
        _PROGRAM = build_program()
    return _PROGRAM


def _bf16(a):
    import ml_dtypes
    return np.ascontiguousarray(np.asarray(a, np.float32)).astype(
        ml_dtypes.bfloat16)


def make_in_maps(x, qkv_w, qkv_b, out_w, out_b, dna_bias):
    x = np.asarray(x, np.float32)
    qkv_w = np.asarray(qkv_w, np.float32)
    qkv_b = np.asarray(qkv_b, np.float32)
    out_w = np.asarray(out_w, np.float32)
    dna_bias = np.asarray(dna_bias, np.float32)

    scale = 1.0 / np.sqrt(HD)
    bias = dna_bias[:T, :T]
    causal = np.tril(np.ones((T, T), np.float32))
    e_t = _bf16((np.exp(bias) * causal).T)

    in_maps = []
    for core in range(N_CORES):
        b, g = divmod(core, 2)
        cols = slice(g * CPC, (g + 1) * CPC)
        wq = qkv_w[0 * DIM:1 * DIM][cols] * scale      # [512, 1024]
        wk = qkv_w[1 * DIM:2 * DIM][cols]
        wv = qkv_w[2 * DIM:3 * DIM][cols]
        in_maps.append({
            "x_t": _bf16(x[b].T),
            "wq_t": _bf16(wq.T),
            "wk_t": _bf16(wk.T),
            "wv_t": _bf16(wv.T),
            "bq": np.ascontiguousarray(
                (qkv_b[0 * DIM:1 * DIM][cols] * scale)[:, None]),
            "bk": np.ascontiguousarray(qkv_b[1 * DIM:2 * DIM][cols][:, None]),
            "bv": np.ascontiguousarray(
                np.broadcast_to(qkv_b[2 * DIM:3 * DIM][cols][None, :],
                                (P, CPC))),
            "wo_t": _bf16(out_w[:, cols].T),
            "e_t": e_t,
        })
    return in_maps


LAST_RESULTS = None


def kernel(x, qkv_w, qkv_b, out_w, out_b, dna_bias, **run_kwargs):
    global LAST_RESULTS
    nc = _get_program()
    in_maps = make_in_maps(x, qkv_w, qkv_b, out_w, out_b, dna_bias)
    res = run_bass_kernel_spmd(nc, in_maps, list(range(N_CORES)), **run_kwargs)
    LAST_RESULTS = res
    out_b = np.asarray(out_b, np.float32)
    out = np.empty((B, T, DIM), np.float32)
    for b in range(B):
        out[b] = res.results[2 * b]["y"] + res.results[2 * b + 1]["y"] + out_b
    return out


# revision 29
# speedup vs baseline: 1.8137x; 1.0227x over previous
"""DNA-structure attention Trainium2 kernel (8-core SPMD).

Reference computation (per batch b):
    qkv = x @ qkv_w.T + qkv_b ; split to q,k,v [H=16 heads, d=64]
    s   = q @ k.T / 8 + dna_bias ; causal mask ; p = softmax(s)
    o   = p @ v ; y = concat_heads(o) @ out_w.T + out_b

Sharding: 8 cores = 4 batches x 2 head-groups (8 heads each).
Each core computes its batch's partial output y_partial = o_g @ out_w[:, cols_g].T;
host sums the two partials per batch and adds out_b.

Per core, one fused pipeline over 512-query chunks qc (matmul operands in
bf16, accumulation in fp32 PSUM):
  projections(qc): QT,KT chunks = W @ x.T (dims-major); V chunk = x @ Wv.T,
      stored per head with an appended ones column (V' [keys, 65]) so that
      p @ V' also yields the softmax denominators (row 64).
  attention(qc), per head: sT [128k, 512q] = K-chunk.T-matmul (fp32 psum);
      p = exp(sT) * E as bf16, where E = exp(dna_bias.T) * causal(0/1) is
      host-precomputed (its zeros implement the causal mask, so no
      max-subtraction and no masking pass); O.T [65, 512q] accumulates over
      key chunks; normalize by the row-64 reciprocal.
  out-projection(qc) -> partial y, summed (plus out_b) on the host.
Causality: only key chunks kc*128 <= qc*512+511 are computed at all.
"""

import sys

if "/opt/trn_rl_repo" not in sys.path:
    sys.path.insert(0, "/opt/trn_rl_repo")

import numpy as np

import concourse.bass as bass
import concourse.mybir as mybir
import concourse.tile as tile
from concourse import bacc
from concourse.bass_utils import run_bass_kernel_spmd

# The axon NTFF-profiling hook lives in trn_agent_boot in this container but
# concourse expects it at antenv.axon_hooks (absent). Register a shim module
# so run_bass_kernel_spmd(trace=True) can capture real HW timings.
if "antenv.axon_hooks" not in sys.modules:
    import types

    def _get_axon_ntff_profile_hook(_cache=[]):
        if not _cache:
            try:
                from trn_agent_boot.trn_boot import _ntff_profile_via_ctypes
                _cache.append(
                    _ntff_profile_via_ctypes("/opt/axon/libaxon_pjrt.so"))
            except Exception:
                _cache.append(None)
        return _cache[0]

    _m = types.ModuleType("antenv.axon_hooks")
    _m.get_axon_ntff_profile_hook = _get_axon_ntff_profile_hook
    sys.modules["antenv.axon_hooks"] = _m

B, T, DIM = 4, 2048, 1024
HEADS = 16
HD = 64  # head dim
N_CORES = 8
HPC = 8            # heads per core
CPC = HPC * HD     # channel slice per core (512)
QC = 512           # query chunk
N_QC = T // QC     # 4
KC = 128           # key chunk (partition dim of scores_T tiles)
N_KC = T // KC     # 16
P = 128

F32 = mybir.dt.float32
BF16 = mybir.dt.bfloat16


def build_program():
    nc = bacc.Bacc("TRN2", target_bir_lowering=False, debug=False,
                   num_devices=N_CORES)

    x_t = nc.declare_dram_parameter("x_t", [DIM, T], BF16, isOutput=False)
    wq_t = nc.declare_dram_parameter("wq_t", [DIM, CPC], BF16, isOutput=False)
    wk_t = nc.declare_dram_parameter("wk_t", [DIM, CPC], BF16, isOutput=False)
    wv_t = nc.declare_dram_parameter("wv_t", [DIM, CPC], BF16, isOutput=False)
    bq = nc.declare_dram_parameter("bq", [CPC, 1], F32, isOutput=False)
    bk = nc.declare_dram_parameter("bk", [CPC, 1], F32, isOutput=False)
    bv = nc.declare_dram_parameter("bv", [P, CPC], F32, isOutput=False)
    wo_t = nc.declare_dram_parameter("wo_t", [CPC, DIM], BF16, isOutput=False)
    e_t = nc.declare_dram_parameter("e_t", [T, T], BF16, isOutput=False)
    y = nc.declare_dram_parameter("y", [T, DIM], F32, isOutput=True)

    NCC = DIM // P    # 8 contraction chunks for qkv projection
    NQD = CPC // P    # 4 dim-chunks of Q/K

    with tile.TileContext(nc) as tc:
        with (
            tc.tile_pool(name="persist", bufs=1) as persist,
            tc.tile_pool(name="wts", bufs=1) as wts,
            tc.tile_pool(name="xw", bufs=10) as xw,
            tc.tile_pool(name="ebuf", bufs=2) as ebuf,
            tc.tile_pool(name="otile", bufs=2) as otile,
            tc.tile_pool(name="wkp", bufs=6) as wkp,
            tc.tile_pool(name="wko", bufs=2) as wko,
            tc.tile_pool(name="wkn", bufs=3) as wkn,
            tc.tile_pool(name="psA", bufs=2, space="PSUM") as psA,
            tc.tile_pool(name="psS", bufs=2, space="PSUM") as psS,
            tc.tile_pool(name="psO", bufs=2, space="PSUM") as psO,
        ):
            # ---------------- persistent SBUF ----------------
            # Q.T [512 dims, T] (matmul rhs, full 128 partitions).
            qt_buf = persist.tile([P, NQD, T], BF16)
            # K.T per head, zero-padded to the full 128 partitions so the
            # scores matmul streams a full-width moving operand: head h
            # occupies partitions (h%2)*64..+63 of slot h, rest stays zero.
            kt_pad = persist.tile([P, HPC, T], BF16)
            # V' per head, padded to 96 columns (32-aligned): cols 0..63 = V,
            # col 64 = ones (softmax denominator trick), cols 65..95 = zero.
            vp_buf = persist.tile([P, N_KC, HPC, 96], BF16)
            bq_sb = persist.tile([P, NQD], F32)
            bk_sb = persist.tile([P, NQD], F32)
            bv_sb = persist.tile([P, CPC], F32)
            wq_sb = wts.tile([P, NCC, CPC], BF16)
            wk_sb = wts.tile([P, NCC, CPC], BF16)
            wv_sb = wts.tile([P, NCC, CPC], BF16)
            wo_sb = wts.tile([P, NQD, DIM], BF16)      # out_w.T slice

            nc.sync.dma_start(bq_sb[:], bq.rearrange("(c p) o -> p (c o)", p=P))
            nc.sync.dma_start(bk_sb[:], bk.rearrange("(c p) o -> p (c o)", p=P))
            nc.sync.dma_start(bv_sb[:], bv[:])
            nc.gpsimd.memset(kt_pad[:], 0.0)
            nc.gpsimd.memset(vp_buf[:], 0.0)
            nc.gpsimd.memset(vp_buf[:, :, :, 64:65], 1.0)

            x_t3 = x_t.rearrange("(c p) t -> p c t", p=P)
            e_t3 = e_t.rearrange("(ko p) t -> p ko t", p=P)
            wq_t3 = wq_t.rearrange("(c p) m -> p c m", p=P)

            QW = 2 * QC               # projection window (1024 tokens)

            def stage_xts(qp):
                tw = slice(qp * QW, (qp + 1) * QW)
                xts = []
                for cc in range(NCC):
                    xt = xw.tile([P, QW], BF16, tag="xt")
                    nc.sync.dma_start(xt[:], x_t3[:, cc, tw])
                    xts.append(xt)
                return xts

            # DMA order matters for the PE start-up stall: the first
            # matmuls need wq chunk cc + x tile cc, in cc order.
            xts0 = None
            for cc in range(NCC):
                nc.sync.dma_start(wq_sb[:, cc, :], wq_t3[:, cc, :])
                if cc == 0:
                    xts0 = stage_xts(0)
            nc.sync.dma_start(wk_sb[:], wk_t.rearrange("(c p) m -> p c m", p=P))
            nc.sync.dma_start(wv_sb[:], wv_t.rearrange("(c p) m -> p c m", p=P))
            nc.sync.dma_start(wo_sb[:], wo_t.rearrange("(c p) d -> p c d", p=P))

            def projections(qp, xts):
                tw = slice(qp * QW, (qp + 1) * QW)
                # Q.T: out [dims 128, 1024] = Wq.T-chunk.T @ x.T-chunk
                for qd in range(NQD):
                    for half in range(2):
                        hq = slice(half * QC, (half + 1) * QC)
                        hw_ = slice(qp * QW + half * QC,
                                    qp * QW + (half + 1) * QC)
                        ps = psA.tile([P, QC], F32, tag="psA")
                        for cc in range(NCC):
                            nc.tensor.matmul(
                                ps[:],
                                wq_sb[:, cc, qd * P:(qd + 1) * P],
                                xts[cc][:, hq],
                                start=(cc == 0), stop=(cc == NCC - 1),
                            )
                        nc.vector.tensor_tensor(
                            qt_buf[:, qd, hw_], ps[:],
                            bq_sb[:, qd:qd + 1].to_broadcast([P, QC]),
                            mybir.AluOpType.add,
                        )
                # K.T into the per-head zero-padded layout
                for hc in range(NQD):
                    for half in range(2):
                        hq = slice(half * QC, (half + 1) * QC)
                        hw_ = slice(qp * QW + half * QC,
                                    qp * QW + (half + 1) * QC)
                        ps = psA.tile([P, QC], F32, tag="psA")
                        for cc in range(NCC):
                            nc.tensor.matmul(
                                ps[:],
                                wk_sb[:, cc, hc * P:(hc + 1) * P],
                                xts[cc][:, hq],
                                start=(cc == 0), stop=(cc == NCC - 1),
                            )
                        for hh in range(2):
                            hsl = slice(hh * HD, hh * HD + HD)
                            nc.vector.tensor_tensor(
                                kt_pad[hsl, 2 * hc + hh, hw_], ps[hsl, :],
                                bk_sb[hsl, hc:hc + 1].to_broadcast([HD, QC]),
                                mybir.AluOpType.add,
                            )
                # V: out [keys 128, 512 dims] = x.T-chunk.T @ Wv.T-chunk
                for ts_ in range(QW // P):
                    kc_idx = qp * (QW // P) + ts_
                    ps = psA.tile([P, QC], F32, tag="psA")
                    for cc in range(NCC):
                        nc.tensor.matmul(
                            ps[:],
                            xts[cc][:, ts_ * P:(ts_ + 1) * P],
                            wv_sb[:, cc, :],
                            start=(cc == 0), stop=(cc == NCC - 1),
                        )
                    nc.vector.tensor_tensor(
                        vp_buf[:, kc_idx, :, 0:HD],
                        ps[:].rearrange("p (h d) -> p h d", d=HD),
                        bv_sb.rearrange("p (h d) -> p h d", d=HD),
                        mybir.AluOpType.add,
                    )

            mult_i = 0  # round-robin counter for the DVE/GpSimd split

            def attention(qc):
                nonlocal mult_i
                tq = slice(qc * QC, (qc + 1) * QC)
                n_kc = (qc + 1) * (QC // KC)      # causal: key chunks used
                e_sb = ebuf.tile([P, N_KC, QC], BF16, tag="e")
                for kp2 in range(n_kc // 2):
                    nc.gpsimd.dma_start(
                        e_sb[:, 2 * kp2:2 * kp2 + 2, :],
                        e_t3[:, 2 * kp2:2 * kp2 + 2, tq])

                ot_buf = otile.tile([P, NQD, QC], BF16, tag="ot")

                def normalize(h, o_ps):
                    # denominators sit in row HD of the accumulator
                    hp = (h % 2) * HD
                    hc = h // 2
                    r_sb = wkn.tile([1, QC], F32, tag="r")
                    nc.vector.reciprocal(r_sb[:], o_ps[HD:HD + 1, :])
                    rr_sb = wkn.tile([HD, QC], F32, tag="rr")
                    nc.gpsimd.partition_broadcast(rr_sb[:], r_sb[:])
                    nc.vector.tensor_tensor(
                        ot_buf[hp:hp + HD, hc, :], o_ps[0:HD, :],
                        rr_sb[:],
                        mybir.AluOpType.mult)

                pending = None        # (h, o_ps) whose normalize is deferred
                for h in range(HPC):
                    hp = (h % 2) * HD        # partition offset inside chunk
                    hc = h // 2              # which 128-dim chunk
                    o_ps = psO.tile([96, QC], F32, tag="o")
                    # Key chunks in pairs: one [128, 2*QC] psum tile
                    # (2 banks) -> single fat exp + single fat mult.
                    # attn@v is emitted one pair behind (software
                    # pipelining) so PE is not blocked on exp+mult latency.
                    n_kp = n_kc // 2
                    p_tiles = [None] * n_kp

                    def emit_av(kp, o_ps=o_ps, h=h, n_kc=n_kc,
                                p_tiles=p_tiles):
                        for j in range(2):
                            kc = 2 * kp + j
                            nc.tensor.matmul(
                                o_ps[:],
                                vp_buf[:, kc, h, :],
                                p_tiles[kp][:, j * QC:(j + 1) * QC],
                                start=(kc == 0), stop=(kc == n_kc - 1),
                            )
                        p_tiles[kp] = None

                    for kp in range(n_kp):
                        s_ps = psS.tile([P, 2 * QC], F32, tag="s")
                        for j in range(2):
                            kc = 2 * kp + j
                            nc.tensor.matmul(
                                s_ps[:, j * QC:(j + 1) * QC],
                                kt_pad[:, h, kc * KC:(kc + 1) * KC],
                                qt_buf[:, hc, tq],
                                start=True, stop=True,
                            )
                        p_sb = wkp.tile([P, 2 * QC], BF16, tag="p")
                        nc.scalar.activation(
                            p_sb[:], s_ps[:],
                            mybir.ActivationFunctionType.Exp)
                        e_pair = e_sb[:, 2 * kp:2 * kp + 2, :] \
                            .rearrange("p a q -> p (a q)")
                        mult_i += 1
                        nc.vector.tensor_tensor(
                            p_sb[:], p_sb[:], e_pair,
                            mybir.AluOpType.mult)
                        p_tiles[kp] = p_sb
                        if kp >= 1:
                            emit_av(kp - 1)
                        # normalize the previous head only after this head's
                        # first multiplies are queued, so the reciprocal does
                        # not block the DVE FIFO in front of them
                        if kp == min(1, n_kp - 1) and pending is not None:
                            normalize(*pending)
                            pending = None
                    emit_av(n_kp - 1)
                    pending = (h, o_ps)
                if pending is not None:
                    normalize(*pending)
                return ot_buf

            def out_projection(qc, ot_buf):
                for m in range(QC // P):
                    for half in range(2):
                        hn = slice(half * QC, (half + 1) * QC)
                        ps = psA.tile([P, QC], F32, tag="psA")
                        for cc in range(NQD):
                            nc.tensor.matmul(
                                ps[:],
                                ot_buf[:, cc, m * P:(m + 1) * P],
                                wo_sb[:, cc, hn],
                                start=(cc == 0), stop=(cc == NQD - 1),
                            )
                        o_sb = wko.tile([P, QC], F32, tag="osb")
                        nc.scalar.copy(o_sb[:], ps[:])
                        nc.sync.dma_start(
                            y[qc * QC + m * P: qc * QC + (m + 1) * P, hn],
                            o_sb[:])

            xts_next = xts0
            for qp in range(N_QC // 2):
                xts = xts_next
                projections(qp, xts)
                ot_buf = attention(2 * qp)
                # prefetch the next window's x tiles only now: their WAR wait
                # (on this window's projections) would otherwise head-block
                # the Sync DMA queue in front of the attention E loads
                if qp + 1 < N_QC // 2:
                    xts_next = stage_xts(qp + 1)
                out_projection(2 * qp, ot_buf)
                ot_buf = attention(2 * qp + 1)
                out_projection(2 * qp + 1, ot_buf)

    nc.finalize()
    return nc


_PROGRAM = None


def _get_program():
    global _PROGRAM
    if _PROGRAM is None:
        _PROGRAM = build_program()
    return _PROGRAM


def _bf16(a):
    import ml_dtypes
    return np.ascontiguousarray(np.asarray(a, np.float32)).astype(
        ml_dtypes.bfloat16)


def make_in_maps(x, qkv_w, qkv_b, out_w, out_b, dna_bias):
    x = np.asarray(x, np.float32)
    qkv_w = np.asarray(qkv_w, np.float32)
    qkv_b = np.asarray(qkv_b, np.float32)
    out_w = np.asarray(out_w, np.float32)
    dna_bias = np.asarray(dna_bias, np.float32)

    scale = 1.0 / np.sqrt(HD)
    bias = dna_bias[:T, :T]
    causal = np.tril(np.ones((T, T), np.float32))
    e_t = _bf16((np.exp(bias) * causal).T)

    in_maps = []
    for core in range(N_CORES):
        b, g = divmod(core, 2)
        cols = slice(g * CPC, (g + 1) * CPC)
        wq = qkv_w[0 * DIM:1 * DIM][cols] * scale      # [512, 1024]
        wk = qkv_w[1 * DIM:2 * DIM][cols]
        wv = qkv_w[2 * DIM:3 * DIM][cols]
        in_maps.append({
            "x_t": _bf16(x[b].T),
            "wq_t": _bf16(wq.T),
            "wk_t": _bf16(wk.T),
            "wv_t": _bf16(wv.T),
            "bq": np.ascontiguousarray(
                (qkv_b[0 * DIM:1 * DIM][cols] * scale)[:, None]),
            "bk": np.ascontiguousarray(qkv_b[1 * DIM:2 * DIM][cols][:, None]),
            "bv": np.ascontiguousarray(
                np.broadcast_to(qkv_b[2 * DIM:3 * DIM][cols][None, :],
                                (P, CPC))),
            "wo_t": _bf16(out_w[:, cols].T),
            "e_t": e_t,
        })
    return in_maps


LAST_RESULTS = None


def kernel(x, qkv_w, qkv_b, out_w, out_b, dna_bias, **run_kwargs):
    global LAST_RESULTS
    nc = _get_program()
    in_maps = make_in_maps(x, qkv_w, qkv_b, out_w, out_b, dna_bias)
    res = run_bass_kernel_spmd(nc, in_maps, list(range(N_CORES)), **run_kwargs)
    LAST_RESULTS = res
    out_b = np.asarray(out_b, np.float32)
    out = np.empty((B, T, DIM), np.float32)
    for b in range(B):
        out[b] = res.results[2 * b]["y"] + res.results[2 * b + 1]["y"] + out_b
    return out


# revision 31
# speedup vs baseline: 1.8213x; 1.0042x over previous
"""DNA-structure attention Trainium2 kernel (8-core SPMD).

Reference computation (per batch b):
    qkv = x @ qkv_w.T + qkv_b ; split to q,k,v [H=16 heads, d=64]
    s   = q @ k.T / 8 + dna_bias ; causal mask ; p = softmax(s)
    o   = p @ v ; y = concat_heads(o) @ out_w.T + out_b

Sharding: 8 cores = 4 batches x 2 head-groups (8 heads each).
Each core computes its batch's partial output y_partial = o_g @ out_w[:, cols_g].T;
host sums the two partials per batch and adds out_b.

Per core, one fused pipeline over 512-query chunks qc (matmul operands in
bf16, accumulation in fp32 PSUM):
  projections(qc): QT,KT chunks = W @ x.T (dims-major); V chunk = x @ Wv.T,
      stored per head with an appended ones column (V' [keys, 65]) so that
      p @ V' also yields the softmax denominators (row 64).
  attention(qc), per head: sT [128k, 512q] = K-chunk.T-matmul (fp32 psum);
      p = exp(sT) * E as bf16, where E = exp(dna_bias.T) * causal(0/1) is
      host-precomputed (its zeros implement the causal mask, so no
      max-subtraction and no masking pass); O.T [65, 512q] accumulates over
      key chunks; normalize by the row-64 reciprocal.
  out-projection(qc) -> partial y, summed (plus out_b) on the host.
Causality: only key chunks kc*128 <= qc*512+511 are computed at all.
"""

import sys

if "/opt/trn_rl_repo" not in sys.path:
    sys.path.insert(0, "/opt/trn_rl_repo")

import numpy as np

import concourse.bass as bass
import concourse.mybir as mybir
import concourse.tile as tile
from concourse import bacc
from concourse.bass_utils import run_bass_kernel_spmd

# The axon NTFF-profiling hook lives in trn_agent_boot in this container but
# concourse expects it at antenv.axon_hooks (absent). Register a shim module
# so run_bass_kernel_spmd(trace=True) can capture real HW timings.
if "antenv.axon_hooks" not in sys.modules:
    import types

    def _get_axon_ntff_profile_hook(_cache=[]):
        if not _cache:
            try:
                from trn_agent_boot.trn_boot import _ntff_profile_via_ctypes
                _cache.append(
                    _ntff_profile_via_ctypes("/opt/axon/libaxon_pjrt.so"))
            except Exception:
                _cache.append(None)
        return _cache[0]

    _m = types.ModuleType("antenv.axon_hooks")
    _m.get_axon_ntff_profile_hook = _get_axon_ntff_profile_hook
    sys.modules["antenv.axon_hooks"] = _m

B, T, DIM = 4, 2048, 1024
HEADS = 16
HD = 64  # head dim
N_CORES = 8
HPC = 8            # heads per core
CPC = HPC * HD     # channel slice per core (512)
QC = 512           # query chunk
N_QC = T // QC     # 4
KC = 128           # key chunk (partition dim of scores_T tiles)
N_KC = T // KC     # 16
P = 128

F32 = mybir.dt.float32
BF16 = mybir.dt.bfloat16


def build_program():
    nc = bacc.Bacc("TRN2", target_bir_lowering=False, debug=False,
                   num_devices=N_CORES)

    x_t = nc.declare_dram_parameter("x_t", [DIM, T], BF16, isOutput=False)
    wq_t = nc.declare_dram_parameter("wq_t", [DIM, CPC], BF16, isOutput=False)
    wk_t = nc.declare_dram_parameter("wk_t", [DIM, CPC], BF16, isOutput=False)
    wv_t = nc.declare_dram_parameter("wv_t", [DIM, CPC], BF16, isOutput=False)
    bq = nc.declare_dram_parameter("bq", [CPC, 1], F32, isOutput=False)
    bk = nc.declare_dram_parameter("bk", [CPC, 1], F32, isOutput=False)
    bv = nc.declare_dram_parameter("bv", [P, CPC], F32, isOutput=False)
    wo_t = nc.declare_dram_parameter("wo_t", [CPC, DIM], BF16, isOutput=False)
    e_t = nc.declare_dram_parameter("e_t", [T, T], BF16, isOutput=False)
    y = nc.declare_dram_parameter("y", [T, DIM], F32, isOutput=True)

    NCC = DIM // P    # 8 contraction chunks for qkv projection
    NQD = CPC // P    # 4 dim-chunks of Q/K

    with tile.TileContext(nc) as tc:
        with (
            tc.tile_pool(name="persist", bufs=1) as persist,
            tc.tile_pool(name="wts", bufs=1) as wts,
            tc.tile_pool(name="xw", bufs=10) as xw,
            tc.tile_pool(name="ebuf", bufs=3) as ebuf,
            tc.tile_pool(name="otile", bufs=2) as otile,
            tc.tile_pool(name="wkp", bufs=6) as wkp,
            tc.tile_pool(name="wko", bufs=2) as wko,
            tc.tile_pool(name="wkn", bufs=2) as wkn,
            tc.tile_pool(name="psA", bufs=2, space="PSUM") as psA,
            tc.tile_pool(name="psS", bufs=2, space="PSUM") as psS,
            tc.tile_pool(name="psO", bufs=2, space="PSUM") as psO,
        ):
            # ---------------- persistent SBUF ----------------
            # Q.T [512 dims, T] (matmul rhs, full 128 partitions).
            qt_buf = persist.tile([P, NQD, T], BF16)
            # K.T per head, zero-padded to the full 128 partitions so the
            # scores matmul streams a full-width moving operand: head h
            # occupies partitions (h%2)*64..+63 of slot h, rest stays zero.
            kt_pad = persist.tile([P, HPC, T], BF16)
            # V' per head, padded to 96 columns (32-aligned): cols 0..63 = V,
            # col 64 = ones (softmax denominator trick), cols 65..95 = zero.
            vp_buf = persist.tile([P, N_KC, HPC, 96], BF16)
            bq_sb = persist.tile([P, NQD], F32)
            bk_sb = persist.tile([P, NQD], F32)
            bv_sb = persist.tile([P, CPC], F32)
            wq_sb = wts.tile([P, NCC, CPC], BF16)
            wk_sb = wts.tile([P, NCC, CPC], BF16)
            wv_sb = wts.tile([P, NCC, CPC], BF16)
            wo_sb = wts.tile([P, NQD, DIM], BF16)      # out_w.T slice

            nc.sync.dma_start(bq_sb[:], bq.rearrange("(c p) o -> p (c o)", p=P))
            nc.sync.dma_start(bk_sb[:], bk.rearrange("(c p) o -> p (c o)", p=P))
            nc.sync.dma_start(bv_sb[:], bv[:])
            nc.gpsimd.memset(kt_pad[:], 0.0)
            nc.gpsimd.memset(vp_buf[:], 0.0)
            nc.gpsimd.memset(vp_buf[:, :, :, 64:65], 1.0)

            x_t3 = x_t.rearrange("(c p) t -> p c t", p=P)
            e_t3 = e_t.rearrange("(ko p) t -> p ko t", p=P)
            wq_t3 = wq_t.rearrange("(c p) m -> p c m", p=P)

            QW = 2 * QC               # projection window (1024 tokens)

            def stage_xts(qp):
                tw = slice(qp * QW, (qp + 1) * QW)
                xts = []
                for cc in range(NCC):
                    xt = xw.tile([P, QW], BF16, tag="xt")
                    nc.sync.dma_start(xt[:], x_t3[:, cc, tw])
                    xts.append(xt)
                return xts

            # DMA order matters for the PE start-up stall: the first
            # matmuls need wq chunk cc + x tile cc, in cc order.
            xts0 = None
            for cc in range(NCC):
                nc.sync.dma_start(wq_sb[:, cc, :], wq_t3[:, cc, :])
                if cc == 0:
                    xts0 = stage_xts(0)
            nc.sync.dma_start(wk_sb[:], wk_t.rearrange("(c p) m -> p c m", p=P))
            nc.sync.dma_start(wv_sb[:], wv_t.rearrange("(c p) m -> p c m", p=P))
            nc.sync.dma_start(wo_sb[:], wo_t.rearrange("(c p) d -> p c d", p=P))

            def projections(qp, xts):
                tw = slice(qp * QW, (qp + 1) * QW)
                # Q.T: out [dims 128, 1024] = Wq.T-chunk.T @ x.T-chunk
                for qd in range(NQD):
                    for half in range(2):
                        hq = slice(half * QC, (half + 1) * QC)
                        hw_ = slice(qp * QW + half * QC,
                                    qp * QW + (half + 1) * QC)
                        ps = psA.tile([P, QC], F32, tag="psA")
                        for cc in range(NCC):
                            nc.tensor.matmul(
                                ps[:],
                                wq_sb[:, cc, qd * P:(qd + 1) * P],
                                xts[cc][:, hq],
                                start=(cc == 0), stop=(cc == NCC - 1),
                            )
                        nc.vector.tensor_tensor(
                            qt_buf[:, qd, hw_], ps[:],
                            bq_sb[:, qd:qd + 1].to_broadcast([P, QC]),
                            mybir.AluOpType.add,
                        )
                # K.T into the per-head zero-padded layout
                for hc in range(NQD):
                    for half in range(2):
                        hq = slice(half * QC, (half + 1) * QC)
                        hw_ = slice(qp * QW + half * QC,
                                    qp * QW + (half + 1) * QC)
                        ps = psA.tile([P, QC], F32, tag="psA")
                        for cc in range(NCC):
                            nc.tensor.matmul(
                                ps[:],
                                wk_sb[:, cc, hc * P:(hc + 1) * P],
                                xts[cc][:, hq],
                                start=(cc == 0), stop=(cc == NCC - 1),
                            )
                        for hh in range(2):
                            hsl = slice(hh * HD, hh * HD + HD)
                            nc.vector.tensor_tensor(
                                kt_pad[hsl, 2 * hc + hh, hw_], ps[hsl, :],
                                bk_sb[hsl, hc:hc + 1].to_broadcast([HD, QC]),
                                mybir.AluOpType.add,
                            )
                # V: out [keys 128, 512 dims] = x.T-chunk.T @ Wv.T-chunk
                for ts_ in range(QW // P):
                    kc_idx = qp * (QW // P) + ts_
                    ps = psA.tile([P, QC], F32, tag="psA")
                    for cc in range(NCC):
                        nc.tensor.matmul(
                            ps[:],
                            xts[cc][:, ts_ * P:(ts_ + 1) * P],
                            wv_sb[:, cc, :],
                            start=(cc == 0), stop=(cc == NCC - 1),
                        )
                    nc.vector.tensor_tensor(
                        vp_buf[:, kc_idx, :, 0:HD],
                        ps[:].rearrange("p (h d) -> p h d", d=HD),
                        bv_sb.rearrange("p (h d) -> p h d", d=HD),
                        mybir.AluOpType.add,
                    )

            mult_i = 0  # round-robin counter for the DVE/GpSimd split

            def attention(qc):
                nonlocal mult_i
                tq = slice(qc * QC, (qc + 1) * QC)
                n_kc = (qc + 1) * (QC // KC)      # causal: key chunks used
                e_sb = ebuf.tile([P, N_KC, QC], BF16, tag="e")
                for kp2 in range(n_kc // 2):
                    nc.gpsimd.dma_start(
                        e_sb[:, 2 * kp2:2 * kp2 + 2, :],
                        e_t3[:, 2 * kp2:2 * kp2 + 2, tq])

                ot_buf = otile.tile([P, NQD, QC], BF16, tag="ot")

                def normalize(h, o_ps):
                    # denominators sit in row HD of the accumulator
                    hp = (h % 2) * HD
                    hc = h // 2
                    r_sb = wkn.tile([1, QC], F32, tag="r")
                    nc.vector.reciprocal(r_sb[:], o_ps[HD:HD + 1, :])
                    rr_sb = wkn.tile([HD, QC], F32, tag="rr")
                    nc.gpsimd.partition_broadcast(rr_sb[:], r_sb[:])
                    nc.vector.tensor_tensor(
                        ot_buf[hp:hp + HD, hc, :], o_ps[0:HD, :],
                        rr_sb[:],
                        mybir.AluOpType.mult)

                pending = None        # (h, o_ps) whose normalize is deferred
                for h in range(HPC):
                    hp = (h % 2) * HD        # partition offset inside chunk
                    hc = h // 2              # which 128-dim chunk
                    o_ps = psO.tile([96, QC], F32, tag="o")
                    # Key chunks in pairs: one [128, 2*QC] psum tile
                    # (2 banks) -> single fat exp + single fat mult.
                    # attn@v is emitted one pair behind (software
                    # pipelining) so PE is not blocked on exp+mult latency.
                    n_kp = n_kc // 2
                    p_tiles = [None] * n_kp

                    def emit_av(kp, o_ps=o_ps, h=h, n_kc=n_kc,
                                p_tiles=p_tiles):
                        for j in range(2):
                            kc = 2 * kp + j
                            nc.tensor.matmul(
                                o_ps[:],
                                vp_buf[:, kc, h, :],
                                p_tiles[kp][:, j * QC:(j + 1) * QC],
                                start=(kc == 0), stop=(kc == n_kc - 1),
                            )
                        p_tiles[kp] = None

                    for kp in range(n_kp):
                        s_ps = psS.tile([P, 2 * QC], F32, tag="s")
                        for j in range(2):
                            kc = 2 * kp + j
                            nc.tensor.matmul(
                                s_ps[:, j * QC:(j + 1) * QC],
                                kt_pad[:, h, kc * KC:(kc + 1) * KC],
                                qt_buf[:, hc, tq],
                                start=True, stop=True,
                            )
                        p_sb = wkp.tile([P, 2 * QC], BF16, tag="p")
                        nc.scalar.activation(
                            p_sb[:], s_ps[:],
                            mybir.ActivationFunctionType.Exp)
                        e_pair = e_sb[:, 2 * kp:2 * kp + 2, :] \
                            .rearrange("p a q -> p (a q)")
                        mult_i += 1
                        nc.vector.tensor_tensor(
                            p_sb[:], p_sb[:], e_pair,
                            mybir.AluOpType.mult)
                        p_tiles[kp] = p_sb
                        if kp >= 1:
                            emit_av(kp - 1)
                        # normalize the previous head only after this head's
                        # first multiplies are queued, so the reciprocal does
                        # not block the DVE FIFO in front of them
                        if kp == min(1, n_kp - 1) and pending is not None:
                            normalize(*pending)
                            pending = None
                    emit_av(n_kp - 1)
                    pending = (h, o_ps)
                if pending is not None:
                    normalize(*pending)
                return ot_buf

            def out_projection(qc, ot_buf):
                for m in range(QC // P):
                    for half in range(2):
                        hn = slice(half * QC, (half + 1) * QC)
                        ps = psA.tile([P, QC], F32, tag="psA")
                        for cc in range(NQD):
                            nc.tensor.matmul(
                                ps[:],
                                ot_buf[:, cc, m * P:(m + 1) * P],
                                wo_sb[:, cc, hn],
                                start=(cc == 0), stop=(cc == NQD - 1),
                            )
                        o_sb = wko.tile([P, QC], F32, tag="osb")
                        nc.scalar.copy(o_sb[:], ps[:])
                        nc.sync.dma_start(
                            y[qc * QC + m * P: qc * QC + (m + 1) * P, hn],
                            o_sb[:])

            xts_next = xts0
            for qp in range(N_QC // 2):
                xts = xts_next
                projections(qp, xts)
                ot_buf = attention(2 * qp)
                # prefetch the next window's x tiles only now: their WAR wait
                # (on this window's projections) would otherwise head-block
                # the Sync DMA queue in front of the attention E loads
                if qp + 1 < N_QC // 2:
                    xts_next = stage_xts(qp + 1)
                out_projection(2 * qp, ot_buf)
                ot_buf = attention(2 * qp + 1)
                out_projection(2 * qp + 1, ot_buf)

    nc.finalize()
    return nc


_PROGRAM = None


def _get_program():
    global _PROGRAM
    if _PROGRAM is None:
        _PROGRAM = build_program()
    return _PROGRAM


def _bf16(a):
    import ml_dtypes
    return np.ascontiguousarray(np.asarray(a, np.float32)).astype(
        ml_dtypes.bfloat16)


def make_in_maps(x, qkv_w, qkv_b, out_w, out_b, dna_bias):
    x = np.asarray(x, np.float32)
    qkv_w = np.asarray(qkv_w, np.float32)
    qkv_b = np.asarray(qkv_b, np.float32)
    out_w = np.asarray(out_w, np.float32)
    dna_bias = np.asarray(dna_bias, np.float32)

    scale = 1.0 / np.sqrt(HD)
    bias = dna_bias[:T, :T]
    causal = np.tril(np.ones((T, T), np.float32))
    e_t = _bf16((np.exp(bias) * causal).T)

    in_maps = []
    for core in range(N_CORES):
        b, g = divmod(core, 2)
        cols = slice(g * CPC, (g + 1) * CPC)
        wq = qkv_w[0 * DIM:1 * DIM][cols] * scale      # [512, 1024]
        wk = qkv_w[1 * DIM:2 * DIM][cols]
        wv = qkv_w[2 * DIM:3 * DIM][cols]
        in_maps.append({
            "x_t": _bf16(x[b].T),
            "wq_t": _bf16(wq.T),
            "wk_t": _bf16(wk.T),
            "wv_t": _bf16(wv.T),
            "bq": np.ascontiguousarray(
                (qkv_b[0 * DIM:1 * DIM][cols] * scale)[:, None]),
            "bk": np.ascontiguousarray(qkv_b[1 * DIM:2 * DIM][cols][:, None]),
            "bv": np.ascontiguousarray(
                np.broadcast_to(qkv_b[2 * DIM:3 * DIM][cols][None, :],
                                (P, CPC))),
            "wo_t": _bf16(out_w[:, cols].T),
            "e_t": e_t,
        })
    return in_maps


LAST_RESULTS = None


def kernel(x, qkv_w, qkv_b, out_w, out_b, dna_bias, **run_kwargs):
    global LAST_RESULTS
    nc = _get_program()
    in_maps = make_in_maps(x, qkv_w, qkv_b, out_w, out_b, dna_bias)
    res = run_bass_kernel_spmd(nc, in_maps, list(range(N_CORES)), **run_kwargs)
    LAST_RESULTS = res
    out_b = np.asarray(out_b, np.float32)
    out = np.empty((B, T, DIM), np.float32)
    for b in range(B):
        out[b] = res.results[2 * b]["y"] + res.results[2 * b + 1]["y"] + out_b
    return out
